# revision 13
# baseline (speedup 1.0000x reference)
"""Multi-head attention (B=4, S=2048, E=1024, H=16, D=64) on 8 trn2 cores.

Sharding: 2D (batch x head-group). Core c handles batch b = c//2 and head
group g = c%2 (8 heads = 512 feature dims). Each core computes a full
[S, E] partial of the output projection for its batch; the host sums the
two group partials per batch and adds the bias.

Per-core device kernel (all fp16/bf16 matmuls, fp32 PSUM accumulation):
  qT = (Wq_loc @ X_q^T)      [512, 2048]  (stored transposed, fp16)
  kT = (Wk_loc @ X_k^T)      [512, 2048]
  v  = X_v @ Wv_loc^T        [2048, 512]  (natural layout + ones column, bf16)
  per head h, per qq-tile (1024), per kk-chunk (128):
    scores^T chunk = kT_h_chunk.T @ qT_h   -> PSUM [128, 1024] f32
    P^T = exp(scores^T): the two qq streams go to different engines so
    neither is the bottleneck (no max subtraction: max |score| ~76 fits
    fp32/bf16 range). Wq is pre-scaled by A_EXP=128/ln2 on the host:
      qt=0: ScalarE activation Exp with scale=1/A_EXP (exact)
      qt=1: VectorE int16(sc + B_EXP) bitcast to bf16 -- the Schraudolph
        2^x bit trick, ~+-3% per element, but softmax normalization
        cancels the shared scale; end-to-end rel err ~5.5e-3 (HW
        measured, budget 2e-2)
    U  += v_aug_chunk.T @ P^T  -> PSUM [65, 1024] f32  (row 64 = softmax denom)
  attnout^T = U[0:64] * broadcast(1/U[64])  -> SBUF fp16
  y = attnout^T.T @ Wo_loc^T  -> [2048, 1024] f32 partial
"""

from contextlib import ExitStack

import numpy as np

S = 2048
E = 1024
F = 512          # local feature dims (8 heads x 64)
HL = 8           # heads per core
D = 64
B = 4
H = 16
NCORES = 8

# exp(x) ~= bf16_frombits(int16(x*128/ln2 + B_EXP)); wq is pre-scaled by
# A_EXP on the host so the device-side op is a single add.
A_EXP = 184.66496523378731
B_EXP = 16248.92

_CACHE = {}


def build_nc(reps: int = 1):
    import concourse.tile as tile
    from concourse import bacc, mybir

    F16 = mybir.dt.float16
    BF16 = mybir.dt.bfloat16
    F32 = mybir.dt.float32
    I16 = mybir.dt.int16
    EXP = mybir.ActivationFunctionType.Exp

    nc = bacc.Bacc(
        "TRN2",
        target_bir_lowering=False,
        debug=False,
        enable_asserts=False,
        num_devices=NCORES,
    )

    xq_d = nc.dram_tensor("xq", [E, S], F16, kind="ExternalInput").ap()
    xk_d = nc.dram_tensor("xk", [E, S], F16, kind="ExternalInput").ap()
    xv_d = nc.dram_tensor("xv", [E, S], F16, kind="ExternalInput").ap()
    wq_d = nc.dram_tensor("wq", [E, F], F16, kind="ExternalInput").ap()
    wk_d = nc.dram_tensor("wk", [E, F], F16, kind="ExternalInput").ap()
    wv_d = nc.dram_tensor("wv", [E, F], F16, kind="ExternalInput").ap()
    wo_d = nc.dram_tensor("wo", [F, E], F16, kind="ExternalInput").ap()
    y_d = nc.dram_tensor("y", [S, E], F32, kind="ExternalOutput").ap()

    with tile.TileContext(nc) as tc, ExitStack() as ctx:
        persist = ctx.enter_context(tc.tile_pool(name="persist", bufs=1))
        xpool = ctx.enter_context(tc.tile_pool(name="xpool", bufs=16))
        ppool = ctx.enter_context(tc.tile_pool(name="ppool", bufs=4))
        ypool = ctx.enter_context(tc.tile_pool(name="ypool", bufs=3))
        smpool = ctx.enter_context(tc.tile_pool(name="smpool", bufs=2))
        ps_s = ctx.enter_context(tc.tile_pool(name="ps_s", bufs=2, space="PSUM"))
        ps_u = ctx.enter_context(tc.tile_pool(name="ps_u", bufs=2, space="PSUM"))

        def body(iv):
            # ---------------- weight/x loads ----------------
            def load_w(dram, pfx, width):
                tiles = []
                nchunks = dram.shape[0] // 128
                for i in range(nchunks):
                    t = persist.tile([128, width], F16, tag=f"{pfx}{i}",
                                     name=f"{pfx}_sb{i}")
                    nc.sync.dma_start(t[:], dram[i * 128:(i + 1) * 128, :])
                    tiles.append(t)
                return tiles

            def load_x(dram, pfx):
                tiles = []
                for eci in range(8):
                    t = xpool.tile([128, S], F16, tag="x", name=f"x{pfx}{eci}")
                    nc.sync.dma_start(t[:], dram[eci * 128:(eci + 1) * 128, :])
                    tiles.append(t)
                return tiles

            # v first (attention depends on all of v); interleave w/x DMAs
            # so the first v-proj matmul starts after ~0.7 MB instead of 5 MB
            wv_sb, xv_sb = [], []
            for eci in range(8):
                t = persist.tile([128, F], F16, tag=f"wv{eci}",
                                 name=f"wv_sb{eci}")
                nc.sync.dma_start(t[:], wv_d[eci * 128:(eci + 1) * 128, :])
                wv_sb.append(t)
                t = xpool.tile([128, S], F16, tag="x", name=f"xv{eci}")
                nc.sync.dma_start(t[:], xv_d[eci * 128:(eci + 1) * 128, :])
                xv_sb.append(t)

            # v with ones column: v_sb[p, tc, h, d] = v[tc*128+p, h*64+d],
            # d=64 column stays 1.0 (softmax denominator trick)
            v_sb = persist.tile([128, 16, HL, D + 1], BF16, tag="v_sb",
                                name="v_sb")
            nc.vector.memset(v_sb[:], 1.0)

            # ---------------- V projection (natural layout) ----------------
            for tci in range(16):
                vp = ps_u.tile([128, F], F32, tag="u", name=f"v_ps{tci}")
                for eci in range(8):
                    nc.tensor.matmul(
                        vp[:],
                        lhsT=xv_sb[eci][:, tci * 128:(tci + 1) * 128],
                        rhs=wv_sb[eci][:],
                        start=(eci == 0),
                        stop=(eci == 7),
                    )
                for h in range(HL):
                    nc.vector.tensor_copy(v_sb[:, tci, h, 0:D],
                                          vp[:, h * D:(h + 1) * D])

            # ---------------- Q/K projection chunks (transposed layout) ----
            wq_sb = load_w(wq_d, "wq", F)
            xq_sb = load_x(xq_d, "q")
            wk_sb = load_w(wk_d, "wk", F)
            xk_sb = load_x(xk_d, "k")
            wo_sb = load_w(wo_d, "wo", E)

            qT_sb = [persist.tile([128, S], F16, tag=f"qT{i}", name=f"qT_sb{i}")
                     for i in range(4)]
            kT_sb = [persist.tile([128, S], F16, tag=f"kT{i}", name=f"kT_sb{i}")
                     for i in range(4)]

            def proj_oc(x_sb, w_sb, ot, oci, pfx):
                for half in range(2):
                    pp = ps_s.tile([128, 1024], F32, tag="s",
                                   name=f"{pfx}p{oci}_{half}")
                    for eci in range(8):
                        for nb in range(2):
                            col = half * 1024 + nb * 512
                            nc.tensor.matmul(
                                pp[:, nb * 512:(nb + 1) * 512],
                                lhsT=w_sb[eci][:, oci * 128:(oci + 1) * 128],
                                rhs=x_sb[eci][:, col:col + 512],
                                start=(eci == 0),
                                stop=(eci == 7),
                            )
                    nc.vector.tensor_copy(
                        ot[:, half * 1024:(half + 1) * 1024], pp[:])

            # attnout^T storage
            aT_sb = [persist.tile([128, S], F16, tag=f"aT{i}", name=f"aT_sb{i}")
                     for i in range(4)]

            # ---------------- attention for one head ----------------
            # Both qq halves (qt=0,1) processed jointly: two interleaved
            # exp streams keep ACT saturated while each stream's scores
            # PSUM tile is effectively single-buffered (4+4 banks total).
            def attn_head(h):
                ch, hh = h // 2, h % 2
                p0, p1 = hh * 64, hh * 64 + 64
                U = [ps_u.tile([65, 1024], F32, tag="u", name=f"U{h}_{qt}")
                     for qt in range(2)]
                prev = [None, None]

                def av(qt, kk, pt):
                    for nb in range(2):
                        nc.tensor.matmul(
                            U[qt][:, nb * 512:(nb + 1) * 512],
                            lhsT=v_sb[:, kk, h, :],
                            rhs=pt[:, nb * 512:(nb + 1) * 512],
                            start=(kk == 0),
                            stop=(kk == 15),
                        )

                for kk in range(16):
                    sc = [None, None]
                    for qt in range(2):
                        s = ps_s.tile([128, 1024], F32, tag="s",
                                      name=f"sc{h}_{qt}_{kk}")
                        for nb in range(2):
                            qcol = qt * 1024 + nb * 512
                            nc.tensor.matmul(
                                s[:, nb * 512:(nb + 1) * 512],
                                lhsT=kT_sb[ch][p0:p1, kk * 128:(kk + 1) * 128],
                                rhs=qT_sb[ch][p0:p1, qcol:qcol + 512],
                                start=True,
                                stop=True,
                            )
                        sc[qt] = s
                        # AV of previous chunk emitted between the two score
                        # streams so the PE always has ready work
                        if prev[qt] is not None:
                            av(qt, kk - 1, prev[qt])
                    ptA = ppool.tile([128, 1024], BF16, tag="p",
                                     name=f"p{h}_0_{kk}")
                    nc.scalar.activation(ptA[:], sc[0][:], EXP,
                                         scale=1.0 / A_EXP)
                    prev[0] = ptA[:]
                    ptB = ppool.tile([128, 1024], I16, tag="p",
                                     name=f"p{h}_1_{kk}")
                    nc.vector.tensor_scalar_add(ptB[:], sc[1][:], B_EXP)
                    prev[1] = ptB[:].bitcast(BF16)
                for qt in range(2):
                    av(qt, 15, prev[qt])

                # normalize: aT = U[0:64] / U[64]
                for qt in range(2):
                    rcp = smpool.tile([1, 1024], F32, tag="rcp",
                                      name=f"rcp{h}_{qt}")
                    nc.vector.reciprocal(rcp[:], U[qt][64:65, :])
                    bc = smpool.tile([64, 1024], F32, tag="bc",
                                     name=f"bc{h}_{qt}")
                    nc.gpsimd.partition_broadcast(bc[:], rcp[:])
                    nc.vector.tensor_mul(
                        aT_sb[ch][p0:p1, qt * 1024:(qt + 1) * 1024],
                        U[qt][0:64, :], bc[:])

            # First q/k chunk upfront, later chunks interleaved at head
            # boundaries (chunk p is needed from head 2p onward).
            proj_oc(xq_sb, wq_sb, qT_sb[0], 0, "q")
            proj_oc(xk_sb, wk_sb, kT_sb[0], 0, "k")
            attn_head(0)
            proj_oc(xq_sb, wq_sb, qT_sb[1], 1, "q")
            attn_head(1)
            proj_oc(xk_sb, wk_sb, kT_sb[1], 1, "k")
            attn_head(2)
            proj_oc(xq_sb, wq_sb, qT_sb[2], 2, "q")
            attn_head(3)
            proj_oc(xk_sb, wk_sb, kT_sb[2], 2, "k")
            attn_head(4)
            proj_oc(xq_sb, wq_sb, qT_sb[3], 3, "q")
            attn_head(5)
            proj_oc(xk_sb, wk_sb, kT_sb[3], 3, "k")
            attn_head(6)
            attn_head(7)

            # ---------------- output projection ----------------
            for tci in range(16):
                yp = ps_u.tile([128, 1024], F32, tag="u", name=f"y_ps{tci}")
                for fc in range(4):
                    for nb in range(2):
                        nc.tensor.matmul(
                            yp[:, nb * 512:(nb + 1) * 512],
                            lhsT=aT_sb[fc][:, tci * 128:(tci + 1) * 128],
                            rhs=wo_sb[fc][:, nb * 512:(nb + 1) * 512],
                            start=(fc == 0),
                            stop=(fc == 3),
                        )
                ysb = ypool.tile([128, 1024], F32, tag="y", name=f"y_sb{tci}")
                # ACT is idle during the output projection; split the PSUM
                # drain copies between ACT and DVE
                if tci % 2 == 0:
                    nc.scalar.copy(ysb[:], yp[:])
                else:
                    nc.vector.tensor_copy(ysb[:], yp[:])
                nc.sync.dma_start(y_d[tci * 128:(tci + 1) * 128, :], ysb[:])

        if reps == 1:
            body(0)
        else:
            with tc.For_i(0, reps, 1) as iv:
                body(iv)

    nc.compile()
    return nc


def make_in_maps(Q, K, V, Wq, Wk, Wv, Wo):
    """Shard + lay out full inputs for the 8 cores."""
    Q = np.asarray(Q, dtype=np.float32)
    K = np.asarray(K, dtype=np.float32)
    V = np.asarray(V, dtype=np.float32)
    # pre-scale Wq by A_EXP: scores come out as A_EXP*s, so the VectorE
    # exp path is a single add and the ScalarE path un-scales for free
    Wq = np.asarray(Wq, dtype=np.float32) * A_EXP
    Wk = np.asarray(Wk, dtype=np.float32)
    Wv = np.asarray(Wv, dtype=np.float32)
    Wo = np.asarray(Wo, dtype=np.float32)

    in_maps = []
    for c in range(NCORES):
        b, g = c // 2, c % 2
        rows = slice(g * F, (g + 1) * F)
        in_maps.append({
            "xq": np.ascontiguousarray(Q[b].T).astype(np.float16),
            "xk": np.ascontiguousarray(K[b].T).astype(np.float16),
            "xv": np.ascontiguousarray(V[b].T).astype(np.float16),
            "wq": np.ascontiguousarray(Wq[rows, :].T).astype(np.float16),
            "wk": np.ascontiguousarray(Wk[rows, :].T).astype(np.float16),
            "wv": np.ascontiguousarray(Wv[rows, :].T).astype(np.float16),
            "wo": np.ascontiguousarray(Wo[:, rows].T).astype(np.float16),
        })
    return in_maps


def combine(results, bo):
    """Sum per-core partials + bias -> full [B, S, E] output."""
    bo = np.asarray(bo, dtype=np.float32)
    y = np.zeros((B, S, E), dtype=np.float32)
    for c in range(NCORES):
        y[c // 2] += results[c]["y"]
    y += bo[None, None, :]
    return y


def kernel(Q, K, V, Wq, Wk, Wv, Wo, bo):
    from concourse.bass_utils import run_bass_kernel_spmd

    if "nc" not in _CACHE:
        _CACHE["nc"] = build_nc(reps=1)
    nc = _CACHE["nc"]
    in_maps = make_in_maps(Q, K, V, Wq, Wk, Wv, Wo)
    res = run_bass_kernel_spmd(nc, in_maps, core_ids=list(range(NCORES)))
    return combine(res.results, bo)


# revision 23
# speedup vs baseline: 1.2051x; 1.2051x over previous
"""Multi-head attention (B=4, S=2048, E=1024, H=16, D=64) on 8 trn2 cores.

Sharding: 2D (batch x head-group). Core c handles batch b = c//2 and head
group g = c%2 (8 heads = 512 feature dims). Each core computes a full
[S, E] partial of the output projection for its batch; the host sums the
two group partials per batch and adds the bias.

Per-core device kernel (all fp16/bf16 matmuls, fp32 PSUM accumulation):
  qT = (Wq_loc @ X_q^T)      [512, 2048]  (stored transposed, fp16)
  kT = (Wk_loc @ X_k^T)      [512, 2048]
  v  = X_v @ Wv_loc^T        [2048, 512]  (natural layout + ones column, bf16)
  per head h, per qq-tile (1024), per kk-chunk (128):
    scores^T chunk = kT_h_chunk.T @ qT_h   -> PSUM [128, 1024] f32
    P^T = exp(scores^T)  (ScalarE, no max subtraction: max score ~76,
                          exp fits fp32/bf16 range)   -> SBUF bf16
    U  += v_aug_chunk.T @ P^T  -> PSUM [65, 1024] f32  (row 64 = softmax denom)
  attnout^T = U[0:64] * broadcast(1/U[64])  -> SBUF fp16
  y = attnout^T.T @ Wo_loc^T  -> [2048, 1024] f32 partial
"""

from contextlib import ExitStack

import numpy as np

S = 2048
E = 1024
F = 512          # local feature dims (8 heads x 64)
HL = 8           # heads per core
D = 64
B = 4
H = 16
NCORES = 8

_CACHE = {}


def build_nc(reps: int = 1):
    import concourse.tile as tile
    from concourse import bacc, mybir

    F16 = mybir.dt.float16
    BF16 = mybir.dt.bfloat16
    F32 = mybir.dt.float32
    EXP = mybir.ActivationFunctionType.Exp

    nc = bacc.Bacc(
        "TRN2",
        target_bir_lowering=False,
        debug=False,
        enable_asserts=False,
        num_devices=NCORES,
    )

    xq_d = nc.dram_tensor("xq", [E, S], F16, kind="ExternalInput").ap()
    xk_d = nc.dram_tensor("xk", [E, S], F16, kind="ExternalInput").ap()
    xv_d = nc.dram_tensor("xv", [E, S], F16, kind="ExternalInput").ap()
    wq_d = nc.dram_tensor("wq", [E, F], F16, kind="ExternalInput").ap()
    wk_d = nc.dram_tensor("wk", [E, F], F16, kind="ExternalInput").ap()
    wv_d = nc.dram_tensor("wv", [E, F], F16, kind="ExternalInput").ap()
    wo_d = nc.dram_tensor("wo", [F, E], F16, kind="ExternalInput").ap()
    # fp16 output halves the y writeback DMA (the tail is DMA-co-limited);
    # partials are O(1) so fp16's 5e-4 relative error is negligible
    y_d = nc.dram_tensor("y", [S, E], F16, kind="ExternalOutput").ap()

    with tile.TileContext(nc) as tc, ExitStack() as ctx:
        persist = ctx.enter_context(tc.tile_pool(name="persist", bufs=1))
        xpool = ctx.enter_context(tc.tile_pool(name="xpool", bufs=16))
        ppool = ctx.enter_context(tc.tile_pool(name="ppool", bufs=6))
        ypool = ctx.enter_context(tc.tile_pool(name="ypool", bufs=3))
        smpool = ctx.enter_context(tc.tile_pool(name="smpool", bufs=2))
        ps_s = ctx.enter_context(tc.tile_pool(name="ps_s", bufs=2, space="PSUM"))
        ps_u = ctx.enter_context(tc.tile_pool(name="ps_u", bufs=2, space="PSUM"))

        def body(iv):
            # ---------------- weight/x loads ----------------
            def load_w(dram, pfx, width):
                tiles = []
                nchunks = dram.shape[0] // 128
                for i in range(nchunks):
                    t = persist.tile([128, width], F16, tag=f"{pfx}{i}",
                                     name=f"{pfx}_sb{i}")
                    nc.sync.dma_start(t[:], dram[i * 128:(i + 1) * 128, :])
                    tiles.append(t)
                return tiles

            def load_x(dram, pfx):
                tiles = []
                for eci in range(8):
                    t = xpool.tile([128, S], F16, tag="x", name=f"x{pfx}{eci}")
                    nc.sync.dma_start(t[:], dram[eci * 128:(eci + 1) * 128, :])
                    tiles.append(t)
                return tiles

            # v first (attention depends on all of v); interleave w/x DMAs
            # so the first v-proj matmul starts after ~0.7 MB instead of 5 MB
            wv_sb, xv_sb = [], []
            for eci in range(8):
                t = persist.tile([128, F], F16, tag=f"wv{eci}",
                                 name=f"wv_sb{eci}")
                nc.sync.dma_start(t[:], wv_d[eci * 128:(eci + 1) * 128, :])
                wv_sb.append(t)
                t = xpool.tile([128, S], F16, tag="x", name=f"xv{eci}")
                nc.sync.dma_start(t[:], xv_d[eci * 128:(eci + 1) * 128, :])
                xv_sb.append(t)

            # v with ones column: v_sb[p, tc, h, d] = v[tc*128+p, h*64+d],
            # d=64 column stays 1.0 (softmax denominator trick)
            v_sb = persist.tile([128, 16, HL, D + 1], BF16, tag="v_sb",
                                name="v_sb")
            nc.vector.memset(v_sb[:], 1.0)

            # ---------------- V projection (natural layout) ----------------
            for tci in range(16):
                vp = ps_u.tile([128, F], F32, tag="u", name=f"v_ps{tci}")
                for eci in range(8):
                    nc.tensor.matmul(
                        vp[:],
                        lhsT=xv_sb[eci][:, tci * 128:(tci + 1) * 128],
                        rhs=wv_sb[eci][:],
                        start=(eci == 0),
                        stop=(eci == 7),
                    )
                for h in range(HL):
                    nc.vector.tensor_copy(v_sb[:, tci, h, 0:D],
                                          vp[:, h * D:(h + 1) * D])

            # ---------------- Q/K projection chunks (transposed layout) ----
            wq_sb = load_w(wq_d, "wq", F)
            xq_sb = load_x(xq_d, "q")
            wk_sb = load_w(wk_d, "wk", F)
            xk_sb = load_x(xk_d, "k")
            wo_sb = load_w(wo_d, "wo", E)

            qT_sb = [persist.tile([128, S], F16, tag=f"qT{i}", name=f"qT_sb{i}")
                     for i in range(4)]
            kT_sb = [persist.tile([128, S], F16, tag=f"kT{i}", name=f"kT_sb{i}")
                     for i in range(4)]

            def proj_half(x_sb, w_sb, ot, oci, half, pfx):
                pp = ps_s.tile([128, 1024], F32, tag="s",
                               name=f"{pfx}p{oci}_{half}")
                for eci in range(8):
                    for nb in range(2):
                        col = half * 1024 + nb * 512
                        nc.tensor.matmul(
                            pp[:, nb * 512:(nb + 1) * 512],
                            lhsT=w_sb[eci][:, oci * 128:(oci + 1) * 128],
                            rhs=x_sb[eci][:, col:col + 512],
                            start=(eci == 0),
                            stop=(eci == 7),
                        )
                nc.vector.tensor_copy(
                    ot[:, half * 1024:(half + 1) * 1024], pp[:])

            def proj_oc(x_sb, w_sb, ot, oci, pfx):
                for half in range(2):
                    proj_half(x_sb, w_sb, ot, oci, half, pfx)

            # attnout^T storage
            aT_sb = [persist.tile([128, S], F16, tag=f"aT{i}", name=f"aT_sb{i}")
                     for i in range(4)]

            # ---------------- attention for one head ----------------
            # Both qq halves (qt=0,1) processed jointly: two interleaved
            # exp streams keep ACT saturated while each stream's scores
            # PSUM tile is effectively single-buffered (4+4 banks total).
            def attn_head(h, extras=()):
                ch, hh = h // 2, h % 2
                p0, p1 = hh * 64, hh * 64 + 64
                U = [ps_u.tile([65, 1024], F32, tag="u", name=f"U{h}_{qt}")
                     for qt in range(2)]
                prev = [None, None]

                def av(qt, kk, pt):
                    for nb in range(2):
                        nc.tensor.matmul(
                            U[qt][:, nb * 512:(nb + 1) * 512],
                            lhsT=v_sb[:, kk, h, :],
                            rhs=pt[:, nb * 512:(nb + 1) * 512],
                            start=(kk == 0),
                            stop=(kk == 15),
                        )

                for kk in range(16):
                    sc = [None, None]
                    for qt in range(2):
                        s = ps_s.tile([128, 1024], F32, tag="s",
                                      name=f"sc{h}_{qt}_{kk}")
                        for nb in range(2):
                            qcol = qt * 1024 + nb * 512
                            nc.tensor.matmul(
                                s[:, nb * 512:(nb + 1) * 512],
                                lhsT=kT_sb[ch][p0:p1, kk * 128:(kk + 1) * 128],
                                rhs=qT_sb[ch][p0:p1, qcol:qcol + 512],
                                start=True,
                                stop=True,
                            )
                        sc[qt] = s
                        # AV of previous chunk emitted between the two score
                        # streams so the PE always has ready work
                        if prev[qt] is not None:
                            av(qt, kk - 1, prev[qt])
                    for qt in range(2):
                        pt = ppool.tile([128, 1024], BF16, tag="p",
                                        name=f"p{h}_{qt}_{kk}")
                        nc.scalar.activation(pt[:], sc[qt][:], EXP)
                        prev[qt] = pt
                    # next q/k projection chunk, emitted as two slices
                    # inside the kk loop: the ps_s rotation hiccup costs one
                    # exp-stream beat instead of a ~7us serial proj window
                    if kk == 5 and len(extras) > 0:
                        extras[0]()
                    if kk == 11 and len(extras) > 1:
                        extras[1]()
                for qt in range(2):
                    av(qt, 15, prev[qt])

                # normalize: aT = U[0:64] / U[64]
                for qt in range(2):
                    rcp = smpool.tile([1, 1024], F32, tag="rcp",
                                      name=f"rcp{h}_{qt}")
                    nc.vector.reciprocal(rcp[:], U[qt][64:65, :])
                    bc = smpool.tile([64, 1024], F32, tag="bc",
                                     name=f"bc{h}_{qt}")
                    nc.gpsimd.partition_broadcast(bc[:], rcp[:])
                    nc.vector.tensor_mul(
                        aT_sb[ch][p0:p1, qt * 1024:(qt + 1) * 1024],
                        U[qt][0:64, :], bc[:])

            # First q/k chunk upfront; chunk i+1 is produced as interleaved
            # slices during heads 2i/2i+1 (needed from head 2i+2 onward).
            def ph(x_sb, w_sb, ot, oci, half, pfx):
                return lambda: proj_half(x_sb, w_sb, ot, oci, half, pfx)

            proj_oc(xq_sb, wq_sb, qT_sb[0], 0, "q")
            proj_oc(xk_sb, wk_sb, kT_sb[0], 0, "k")
            attn_head(0, (ph(xq_sb, wq_sb, qT_sb[1], 1, 0, "q"),
                          ph(xq_sb, wq_sb, qT_sb[1], 1, 1, "q")))
            attn_head(1, (ph(xk_sb, wk_sb, kT_sb[1], 1, 0, "k"),
                          ph(xk_sb, wk_sb, kT_sb[1], 1, 1, "k")))
            attn_head(2, (ph(xq_sb, wq_sb, qT_sb[2], 2, 0, "q"),
                          ph(xq_sb, wq_sb, qT_sb[2], 2, 1, "q")))
            attn_head(3, (ph(xk_sb, wk_sb, kT_sb[2], 2, 0, "k"),
                          ph(xk_sb, wk_sb, kT_sb[2], 2, 1, "k")))
            attn_head(4, (ph(xq_sb, wq_sb, qT_sb[3], 3, 0, "q"),
                          ph(xq_sb, wq_sb, qT_sb[3], 3, 1, "q")))
            attn_head(5, (ph(xk_sb, wk_sb, kT_sb[3], 3, 0, "k"),
                          ph(xk_sb, wk_sb, kT_sb[3], 3, 1, "k")))
            attn_head(6)
            attn_head(7)

            # ---------------- output projection ----------------
            for tci in range(16):
                yp = ps_u.tile([128, 1024], F32, tag="u", name=f"y_ps{tci}")
                for fc in range(4):
                    for nb in range(2):
                        nc.tensor.matmul(
                            yp[:, nb * 512:(nb + 1) * 512],
                            lhsT=aT_sb[fc][:, tci * 128:(tci + 1) * 128],
                            rhs=wo_sb[fc][:, nb * 512:(nb + 1) * 512],
                            start=(fc == 0),
                            stop=(fc == 3),
                        )
                ysb = ypool.tile([128, 1024], F16, tag="y", name=f"y_sb{tci}")
                # ACT is idle during the output projection; split the PSUM
                # drain copies between ACT and DVE
                if tci % 2 == 0:
                    nc.scalar.copy(ysb[:], yp[:])
                else:
                    nc.vector.tensor_copy(ysb[:], yp[:])
                nc.sync.dma_start(y_d[tci * 128:(tci + 1) * 128, :], ysb[:])

        if reps == 1:
            body(0)
        else:
            with tc.For_i(0, reps, 1) as iv:
                body(iv)

    nc.compile()
    return nc


def make_in_maps(Q, K, V, Wq, Wk, Wv, Wo):
    """Shard + lay out full inputs for the 8 cores."""
    Q = np.asarray(Q, dtype=np.float32)
    K = np.asarray(K, dtype=np.float32)
    V = np.asarray(V, dtype=np.float32)
    Wq = np.asarray(Wq, dtype=np.float32)
    Wk = np.asarray(Wk, dtype=np.float32)
    Wv = np.asarray(Wv, dtype=np.float32)
    Wo = np.asarray(Wo, dtype=np.float32)

    in_maps = []
    for c in range(NCORES):
        b, g = c // 2, c % 2
        rows = slice(g * F, (g + 1) * F)
        in_maps.append({
            "xq": np.ascontiguousarray(Q[b].T).astype(np.float16),
            "xk": np.ascontiguousarray(K[b].T).astype(np.float16),
            "xv": np.ascontiguousarray(V[b].T).astype(np.float16),
            "wq": np.ascontiguousarray(Wq[rows, :].T).astype(np.float16),
            "wk": np.ascontiguousarray(Wk[rows, :].T).astype(np.float16),
            "wv": np.ascontiguousarray(Wv[rows, :].T).astype(np.float16),
            "wo": np.ascontiguousarray(Wo[:, rows].T).astype(np.float16),
        })
    return in_maps


def combine(results, bo):
    """Sum per-core partials + bias -> full [B, S, E] output."""
    bo = np.asarray(bo, dtype=np.float32)
    y = np.zeros((B, S, E), dtype=np.float32)
    for c in range(NCORES):
        y[c // 2] += results[c]["y"].astype(np.float32)
    y += bo[None, None, :]
    return y


def kernel(Q, K, V, Wq, Wk, Wv, Wo, bo):
    from concourse.bass_utils import run_bass_kernel_spmd

    if "nc" not in _CACHE:
        _CACHE["nc"] = build_nc(reps=1)
    nc = _CACHE["nc"]
    in_maps = make_in_maps(Q, K, V, Wq, Wk, Wv, Wo)
    res = run_bass_kernel_spmd(nc, in_maps, core_ids=list(range(NCORES)))
    return combine(res.results, bo)
